# revision 32
# baseline (speedup 1.0000x reference)
"""Trainium2 Bass kernel for pre-LN multi-head GQA attention (B=2, S=2048, H=2048,
NH=16, D=128, NKV=4, causal, RoPE).

Sharding: 8 cores = 2 batches x 4 KV groups. Core c handles batch c//4 and KV head
c%4 (its 4 query heads; Wq/Wk/Wv column-sharded by head, Wo row-sharded). Each core
computes a partial output [S, H] (bf16); the host sums the 4 per-batch partials.

v2 dataflow (per core, per 512-wide s-chunk):
  Host folds ln_gamma into the weights and the per-token rstd into the streamed
  activations (x' = rstd * x, transposed), then splits both x' and the weights
  into fp8e4m3 hi + unscaled-residual lo parts (one shared PSUM scale). QKV
  projections run as fp8 DoubleRow matmuls (256-deep contraction, 0.5 cyc/row),
  3 term streams: hi*hi, lo*hi, hi*lo. The LayerNorm mean correction is a bf16
  rank-1 matmul (wsum^T x brow, brow = -mu*rstd) in the same PSUM group.
  RoPE on Q^T/K^T in bf16 (PE rotation matmul + DVE/Pool elementwise); V^T is
  transposed to V via PE (bf16). Attention per head in the k-partition layout:
  logits^T = K^T.T Q^T (bf16 -> fp32 PSUM), exp on ACT -> e bf16, causal mask via
  affine_select on diagonal blocks, denominators as per-s-block column matmuls
  (lhsT = e block, rhs = ones column -> [128,1] PSUM cols, nearly free),
  ctx^T = V.T e (bf16). Reciprocal on tiny columns; broadcast back to rows via
  transpose + flatten-DMA + ones-column rank-1. ctx is evicted unnormalized
  (bf16), then normalized+quantized to fp8 hi/lo. Output projection is fp8
  DoubleRow over head pairs (3 terms), evicted to bf16 stages (ACT/DVE/Pool
  round-robin) and DMAd out.
"""

import sys

for p in ("/opt/trn_rl_repo",):
    if p not in sys.path:
        sys.path.append(p)

import numpy as np
import ml_dtypes

import concourse.bass as bass
import concourse.tile as tile
from concourse import bacc
from concourse import mybir
from concourse.masks import make_identity

F32 = mybir.dt.float32
BF16 = mybir.dt.bfloat16
FP8 = mybir.dt.float8e4
ALU = mybir.AluOpType
ACTF = mybir.ActivationFunctionType
DR = mybir.MatmulPerfMode.DoubleRow

NPF8 = ml_dtypes.float8_e4m3
NPBF = ml_dtypes.bfloat16

B, S, H = 2, 2048, 2048
NH, D, NKV = 16, 128, 4
G = NH // NKV  # query heads per KV head (= heads per core)
EPS = 1e-6
MIN_WIN, MAX_WIN = 1.0, 10000.0
SCALE = 1.0 / float(np.sqrt(np.float32(D)))
CHUNK = 512
NCH = S // CHUNK  # 4
NJP = H // 256  # 8 h-chunk pairs
SX = 2.0
SW = 64.0
SCTX = 8.0
SWO = 64.0
INV_PROJ = 1.0 / (SX * SW)
INV_OUT = 1.0 / (SCTX * SWO)


def build_program(has_bias: bool) -> bass.Bass:
    nc = bacc.Bacc(
        "TRN2",
        target_bir_lowering=False,
        debug=False,
        enable_asserts=False,
        num_devices=8,
    )
    xh_d = nc.dram_tensor("xh", [128, NJP, 2, S], FP8, kind="ExternalInput").ap()
    xl_d = nc.dram_tensor("xl", [128, NJP, 2, S], FP8, kind="ExternalInput").ap()
    wqh_d = nc.dram_tensor("wqh", [128, NJP, 2, G * D], FP8, kind="ExternalInput").ap()
    wql_d = nc.dram_tensor("wql", [128, NJP, 2, G * D], FP8, kind="ExternalInput").ap()
    wkh_d = nc.dram_tensor("wkh", [128, NJP, 2, D], FP8, kind="ExternalInput").ap()
    wkl_d = nc.dram_tensor("wkl", [128, NJP, 2, D], FP8, kind="ExternalInput").ap()
    wvh_d = nc.dram_tensor("wvh", [128, NJP, 2, D], FP8, kind="ExternalInput").ap()
    wvl_d = nc.dram_tensor("wvl", [128, NJP, 2, D], FP8, kind="ExternalInput").ap()
    woh_d = nc.dram_tensor("woh", [128, 2, 2, 4, 512], FP8, kind="ExternalInput").ap()
    wol_d = nc.dram_tensor("wol", [128, 2, 2, 4, 512], FP8, kind="ExternalInput").ap()
    wsq_d = nc.dram_tensor("wsq", [1, G * D], BF16, kind="ExternalInput").ap()
    wsk_d = nc.dram_tensor("wsk", [1, D], BF16, kind="ExternalInput").ap()
    wsv_d = nc.dram_tensor("wsv", [1, D], BF16, kind="ExternalInput").ap()
    brow_d = nc.dram_tensor("brow", [1, S], BF16, kind="ExternalInput").ap()
    bq_d = nc.dram_tensor("bqr", [1, G * D], BF16, kind="ExternalInput").ap()
    bk_d = nc.dram_tensor("bkr", [1, D], BF16, kind="ExternalInput").ap()
    bv_d = nc.dram_tensor("bvr", [1, D], BF16, kind="ExternalInput").ap()
    ones_row_d = nc.dram_tensor("ones_row", [1, S], BF16, kind="ExternalInput").ap()
    cos_d = nc.dram_tensor("cos_t", [128, S], BF16, kind="ExternalInput").ap()
    sin_d = nc.dram_tensor("sin_t", [128, S], BF16, kind="ExternalInput").ap()
    prot_d = nc.dram_tensor("prot", [128, 128], BF16, kind="ExternalInput").ap()
    ones_d = nc.dram_tensor("onesc", [128, 128], BF16, kind="ExternalInput").ap()
    outp = nc.dram_tensor("outp", [S, H], BF16, kind="ExternalOutput").ap()

    with tile.TileContext(nc) as tc:
        with (
            tc.tile_pool(name="singles", bufs=1) as singles,
            tc.tile_pool(name="xp", bufs=2) as xp,
            tc.tile_pool(name="work", bufs=4) as work,
            tc.tile_pool(name="qp", bufs=6) as qp,
            tc.tile_pool(name="ep", bufs=6) as ep,
            tc.tile_pool(name="cp", bufs=2) as cp,
            tc.tile_pool(name="cup", bufs=5) as cup,
            tc.tile_pool(name="stg", bufs=6) as stg,
            # PSUM budget (16KB/partition): psL 3x2KB + psA 2x2KB + psC 1x2KB
            #  + psD 1x2KB(64B used) + psM 1x2KB = 16KB
            tc.tile_pool(name="psL", bufs=3, space="PSUM") as psL,
            tc.tile_pool(name="psA", bufs=2, space="PSUM") as psA,
            tc.tile_pool(name="psC", bufs=1, space="PSUM") as psC,
            tc.tile_pool(name="psD", bufs=1, space="PSUM") as psD,
            tc.tile_pool(name="psM", bufs=1, space="PSUM") as psM,
        ):
            # ---- resident constants/weights ----
            ones_sb = singles.tile([128, 128], BF16)
            nc.scalar.dma_start(ones_sb, ones_d)
            prot_sb = singles.tile([128, 128], BF16)
            nc.scalar.dma_start(prot_sb, prot_d)
            wkh_sb = singles.tile([128, NJP, 2, D], FP8)
            nc.gpsimd.dma_start(wkh_sb, wkh_d)
            wvh_sb = singles.tile([128, NJP, 2, D], FP8)
            nc.scalar.dma_start(wvh_sb, wvh_d)
            wkl_sb = singles.tile([128, NJP, 2, D], FP8)
            nc.scalar.dma_start(wkl_sb, wkl_d)
            wvl_sb = singles.tile([128, NJP, 2, D], FP8)
            nc.scalar.dma_start(wvl_sb, wvl_d)
            wqh_sb = singles.tile([128, NJP, 2, G * D], FP8)
            nc.gpsimd.dma_start(wqh_sb[:, 0:4], wqh_d[:, 0:4])
            nc.gpsimd.dma_start(wqh_sb[:, 4:8], wqh_d[:, 4:8])
            wql_sb = singles.tile([128, NJP, 2, G * D], FP8)
            nc.scalar.dma_start(wql_sb[:, 0:4], wql_d[:, 0:4])
            nc.scalar.dma_start(wql_sb[:, 4:8], wql_d[:, 4:8])
            cos_sb = singles.tile([128, S], BF16)
            nc.gpsimd.dma_start(cos_sb, cos_d)
            sin_sb = singles.tile([128, S], BF16)
            nc.gpsimd.dma_start(sin_sb, sin_d)
            woh_sb = singles.tile([128, 2, 2, 4, 512], FP8)
            nc.gpsimd.dma_start(woh_sb, woh_d)
            wol_sb = singles.tile([128, 2, 2, 4, 512], FP8)
            nc.gpsimd.dma_start(wol_sb, wol_d)
            wsq_sb = singles.tile([1, G * D], BF16)
            nc.scalar.dma_start(wsq_sb, wsq_d)
            wsk_sb = singles.tile([1, D], BF16)
            nc.scalar.dma_start(wsk_sb, wsk_d)
            wsv_sb = singles.tile([1, D], BF16)
            nc.scalar.dma_start(wsv_sb, wsv_d)
            brow_sb = singles.tile([1, S], BF16)
            nc.scalar.dma_start(brow_sb, brow_d)
            if has_bias:
                bq_sb = singles.tile([1, G * D], BF16)
                nc.scalar.dma_start(bq_sb, bq_d)
                bk_sb = singles.tile([1, D], BF16)
                nc.scalar.dma_start(bk_sb, bk_d)
                bv_sb = singles.tile([1, D], BF16)
                nc.scalar.dma_start(bv_sb, bv_d)
                onesr_sb = singles.tile([1, S], BF16)
                nc.scalar.dma_start(onesr_sb, ones_row_d)
            identf = singles.tile([128, 128], F32)
            make_identity(nc, identf)
            identb = singles.tile([128, 128], BF16)
            nc.vector.tensor_copy(identb, identf)
            kT_sb = singles.tile([128, S], BF16)  # roped K^T
            v_sb = singles.tile([128, S // 128, D], BF16)  # V natural

            def proj_group(pt, wh, wl, xh, xl, ws, bias, sl):
                """Accumulate one projection tile [128, CHUNK] into psum pt."""
                n = 0
                last = 3 * NJP + 1 + (1 if bias is not None else 0)
                for term in range(3):
                    w_, x_ = ((wh, xh), (wh, xl), (wl, xh))[term]
                    for j in range(NJP):
                        n += 1
                        nc.tensor.matmul(
                            pt,
                            w_[:, j],
                            x_[:, j],
                            start=(n == 1),
                            stop=False,
                            perf_mode=DR,
                            skip_group_check=(n > 1),
                        )
                n += 1
                nc.tensor.matmul(
                    pt, ws, brow_sb[:, sl], start=False, stop=(n == last),
                    skip_group_check=(n != last),
                )
                if bias is not None:
                    nc.tensor.matmul(
                        pt, bias, onesr_sb[:, sl], start=False, stop=True,
                    )

            def rope(out, raw, cos_c, sin_c):
                """out(bf16) = raw*cos + (P_rot@raw)*sin for one [128, CHUNK]."""
                rps = psL.tile([128, CHUNK], F32, tag="pl", name="rot")
                nc.tensor.matmul(rps, prot_sb, raw, start=True, stop=True)
                tmp = work.tile([128, CHUNK], BF16, tag="ropetmp")
                nc.vector.tensor_mul(tmp, rps, sin_c)
                rc = work.tile([128, CHUNK], BF16, tag="ropecos")
                nc.gpsimd.tensor_mul(rc, raw, cos_c)
                nc.vector.tensor_add(out, rc, tmp)

            def emit_proj(qb):
                """QKV projections + rope for chunk qb; returns q tiles."""
                sl = slice(qb * CHUNK, (qb + 1) * CHUNK)
                cos_c = cos_sb[:, sl]
                sin_c = sin_sb[:, sl]

                xh_sb = xp.tile([128, NJP, 2, CHUNK], FP8, tag="xh")
                for jq in range(4):
                    nc.sync.dma_start(xh_sb[:, 2 * jq:2 * jq + 2],
                                      xh_d[:, 2 * jq:2 * jq + 2, :, sl])
                xl_sb = xp.tile([128, NJP, 2, CHUNK], FP8, tag="xl")
                for jq in range(4):
                    nc.sync.dma_start(xl_sb[:, 2 * jq:2 * jq + 2],
                                      xl_d[:, 2 * jq:2 * jq + 2, :, sl])

                pk = psA.tile([128, CHUNK], F32, tag="proj", name="pk")
                proj_group(pk, wkh_sb, wkl_sb, xh_sb, xl_sb, wsk_sb,
                           bk_sb if has_bias else None, sl)
                pv = psA.tile([128, CHUNK], F32, tag="proj", name="pv")
                proj_group(pv, wvh_sb, wvl_sb, xh_sb, xl_sb, wsv_sb,
                           bv_sb if has_bias else None, sl)

                kraw = work.tile([128, CHUNK], BF16, tag="kraw")
                nc.scalar.mul(kraw, pk, INV_PROJ)
                rope(kT_sb[:, sl], kraw, cos_c, sin_c)

                vt = work.tile([128, CHUNK], BF16, tag="vt")
                nc.scalar.mul(vt, pv, INV_PROJ)
                ptv = psL.tile([128, 4, 128], BF16, tag="pl", name="vtr")
                for m in range(4):
                    nc.tensor.transpose(
                        ptv[:, m, :], vt[:, m * 128:(m + 1) * 128], identb
                    )
                nc.vector.tensor_scalar_mul(
                    v_sb[:, qb * 4: qb * 4 + 4, :], ptv, 1.0
                )

                qts = []
                for g_ in range(G):
                    cs = slice(g_ * D, (g_ + 1) * D)
                    pq = psA.tile([128, CHUNK], F32, tag="proj", name=f"pq{g_}")
                    proj_group(pq, wqh_sb[:, :, :, cs], wql_sb[:, :, :, cs],
                               xh_sb, xl_sb, wsq_sb[:, cs],
                               bq_sb[:, cs] if has_bias else None, sl)
                    qraw = work.tile([128, CHUNK], BF16, tag="qraw")
                    nc.scalar.mul(qraw, pq, INV_PROJ)
                    q_g = qp.tile([128, CHUNK], BF16, tag="q")
                    rope(q_g, qraw, cos_c, sin_c)
                    qts.append(q_g)
                return qts

            def emit_attention(qb, qts):
                kmax = 4 * (qb + 1)
                recs = []
                ctxs_u = []
                for g_ in range(G):
                    pctx = psC.tile([128, CHUNK], F32, tag="ctx")
                    den_ps = psD.tile([128, 4], F32, tag="den")
                    nden = 0
                    tden = sum(4 - max(0, kb - 4 * qb) for kb in range(kmax))
                    for kb in range(kmax):
                        i_d = max(0, kb - 4 * qb)  # first valid s-block
                        vs = slice(i_d * 128, CHUNK)
                        pl = psL.tile([128, CHUNK], F32, tag="pl")
                        nc.tensor.matmul(
                            pl[:, vs],
                            kT_sb[:, kb * 128:(kb + 1) * 128],
                            qts[g_][:, vs],
                            start=True,
                            stop=True,
                        )
                        e = ep.tile([128, CHUNK], BF16, tag="e")
                        nc.scalar.activation(e[:, vs], pl[:, vs], ACTF.Exp,
                                             scale=SCALE)
                        if kb >= 4 * qb:
                            # causal triangle within the diagonal 128-block
                            ds = slice(i_d * 128, (i_d + 1) * 128)
                            nc.gpsimd.affine_select(
                                out=e[:, ds],
                                in_=e[:, ds],
                                compare_op=ALU.is_ge,
                                fill=0.0,
                                base=0,
                                pattern=[[1, 128]],
                                channel_multiplier=-1,
                            )
                        for m in range(i_d, 4):
                            nden += 1
                            nc.tensor.matmul(
                                den_ps[:, m:m + 1],
                                e[:, m * 128:(m + 1) * 128],
                                ones_sb[:, 0:1],
                                start=(nden == 1),
                                stop=(nden == tden),
                                skip_group_check=(1 < nden < tden),
                            )
                        nc.tensor.matmul(
                            pctx[:, vs],
                            v_sb[:, kb, :],
                            e[:, vs],
                            start=(kb == 0),
                            stop=(kb == kmax - 1),
                        )
                    # reciprocal chain: cols -> row -> flatten
                    rcol = work.tile([128, 4], F32, tag="rcol")
                    nc.vector.reciprocal(rcol, den_ps)
                    rcolb = work.tile([128, 4], BF16, tag="rcolb")
                    nc.scalar.copy(rcolb, rcol)
                    prt = psL.tile([4, 128], BF16, tag="pl", name="rect")
                    nc.tensor.transpose(prt, rcolb, identb)
                    rrow4 = work.tile([4, 128], BF16, tag="rrow4")
                    nc.scalar.copy(rrow4, prt)
                    rflat = work.tile([1, 4, 128], BF16, tag="rflat")
                    nc.sync.dma_start(rflat, rrow4)
                    recs.append(rflat)
                    cu = cup.tile([128, CHUNK], BF16, tag="cu")
                    nc.vector.tensor_scalar_mul(cu, pctx, 1.0)
                    ctxs_u.append(cu)

                # normalize + quantize ctx -> fp8 hi/lo
                ctx_hi = cp.tile([128, G, CHUNK], FP8, tag="chi")
                ctx_lo = cp.tile([128, G, CHUNK], FP8, tag="clo")
                for g_ in range(G):
                    prb = psL.tile([128, CHUNK], F32, tag="pl", name="prb")
                    nc.tensor.matmul(prb, ones_sb[0:1, :], recs[g_][0:1],
                                     start=True, stop=True)
                    cbf = cp.tile([128, CHUNK], BF16, tag="cbf")
                    nc.vector.scalar_tensor_tensor(
                        out=cbf, in0=ctxs_u[g_], scalar=SCTX, in1=prb,
                        op0=ALU.mult, op1=ALU.mult,
                    )
                    nc.scalar.copy(ctx_hi[:, g_, :], cbf)
                    nc.gpsimd.tensor_sub(ctx_lo[:, g_, :], cbf,
                                         ctx_hi[:, g_, :])
                return ctx_hi, ctx_lo

            def emit_outproj(qb, ctx_hi, ctx_lo):
                for sm in range(4):
                    row = slice(qb * CHUNK + sm * 128, qb * CHUNK + (sm + 1) * 128)
                    ss = slice(sm * 128, (sm + 1) * 128)
                    for cb in range(4):
                        if qb == NCH - 1 and (sm * 4 + cb) % 2 == 1:
                            po = psA.tile([128, CHUNK], F32, tag="proj",
                                          name="po")
                        else:
                            po = psM.tile([128, 512], F32, tag="po", name="po")
                        n = 0  # 6 DR terms
                        for term in range(3):
                            ch, wo_ = ((ctx_hi, woh_sb), (ctx_lo, woh_sb),
                                       (ctx_hi, wol_sb))[term]
                            for gp in range(2):
                                n += 1
                                nc.tensor.matmul(
                                    po,
                                    ch[:, 2 * gp:2 * gp + 2, ss],
                                    wo_[:, gp, :, cb, :],
                                    start=(n == 1),
                                    stop=(n == 6),
                                    perf_mode=DR,
                                    skip_group_check=(1 < n < 6),
                                )  # gp-major
                        stage = stg.tile([128, 512], BF16, tag="stage")
                        nc.vector.tensor_scalar_mul(stage, po, INV_OUT)
                        nc.sync.dma_start(
                            outp[row, cb * 512:(cb + 1) * 512], stage
                        )

            # software-pipelined chunk loop: attention(qb) emits before
            # proj(qb+1), and outproj(qb) after it, so next-chunk projection
            # matmuls fill attention's exp-latency gaps and outproj fills
            # the following chunk's.
            qts = emit_proj(0)
            for qb in range(NCH):
                ctx_hi, ctx_lo = emit_attention(qb, qts)
                if qb + 1 < NCH:
                    qts = emit_proj(qb + 1)
                emit_outproj(qb, ctx_hi, ctx_lo)
    nc.compile()
    return nc


_PROGRAMS: dict[bool, bass.Bass] = {}


def get_program(has_bias: bool) -> bass.Bass:
    if has_bias not in _PROGRAMS:
        _PROGRAMS[has_bias] = build_program(has_bias)
    return _PROGRAMS[has_bias]


def _split_fp8(t, scale):
    """Return (hi, lo) fp8e4m3 arrays: t*scale = hi + lo (unscaled residual)."""
    ts = np.asarray(t, np.float32) * scale
    assert np.abs(ts).max() < 239.0, f"fp8 overflow {np.abs(ts).max()}"
    hi = ts.astype(NPF8)
    lo = (ts - hi.astype(np.float32)).astype(NPF8)
    return hi, lo


def _pack_pairs(w):
    """[H, M] -> [128, NJP, 2, M]: row (2j+i)*128+p -> [p, j, i, :]."""
    Hh, M = w.shape
    return np.ascontiguousarray(
        w.reshape(NJP, 2, 128, M).transpose(2, 0, 1, 3)
    )


def make_in_maps(x, ln_gamma, ln_beta, Wq, Wk, Wv, Wo):
    x = np.asarray(x, np.float64)
    g = np.asarray(ln_gamma, np.float64)
    be = np.asarray(ln_beta, np.float64)
    Wq = np.asarray(Wq, np.float64)
    Wk = np.asarray(Wk, np.float64)
    Wv = np.asarray(Wv, np.float64)
    Wo = np.asarray(Wo, np.float64)

    Wqg = Wq * g[:, None]
    Wkg = Wk * g[:, None]
    Wvg = Wv * g[:, None]
    bq_full = be @ Wq
    bk_full = be @ Wk
    bv_full = be @ Wv
    has_bias = bool(np.any(be != 0.0))

    # host LN stats
    mu = x.mean(-1, keepdims=True)  # [B, S, 1]
    var = ((x - mu) ** 2).mean(-1, keepdims=True)
    rstd = 1.0 / np.sqrt(var + EPS)
    xs = x * rstd  # x' = rstd * x
    brow = (-mu[..., 0] * rstd[..., 0]) * SX  # [B, S]

    # rope tables (halves duplicated)
    half = D // 2
    ts = MIN_WIN * (MAX_WIN / MIN_WIN) ** (
        2.0 * np.arange(half, dtype=np.float64) / D
    )
    ang = np.arange(S, dtype=np.float64)[None, :] / ts[:, None]
    cos_t = np.concatenate([np.cos(ang), np.cos(ang)], axis=0)
    sin_t = np.concatenate([np.sin(ang), np.sin(ang)], axis=0)

    prot = np.zeros((128, 128), np.float32)
    for m in range(half):
        prot[m + half, m] = -1.0
        prot[m, m + half] = 1.0

    # per-batch x' streams: [128, NJP, 2, S]
    xh_b, xl_b = [], []
    for b in range(B):
        xT = np.ascontiguousarray(xs[b].T)  # [H, S]
        hi, lo = _split_fp8(xT, SX)
        xh_b.append(_pack_pairs(hi.astype(np.float32)).astype(NPF8))
        xl_b.append(_pack_pairs(lo.astype(np.float32)).astype(NPF8))

    def wpack(Wg):
        hi, lo = _split_fp8(Wg, SW)
        return (_pack_pairs(hi.astype(np.float32)).astype(NPF8),
                _pack_pairs(lo.astype(np.float32)).astype(NPF8))

    in_maps = []
    for c in range(8):
        b, h = divmod(c, NKV)
        qs = slice(h * G * D, (h + 1) * G * D)
        ks = slice(h * D, (h + 1) * D)
        wqh, wql = wpack(Wqg[:, qs])
        wkh, wkl = wpack(Wkg[:, ks])
        wvh, wvl = wpack(Wvg[:, ks])
        # Wo rows for this core's 4 heads: [G*D, H]; head-pair packed:
        # [128(d), gp, i, cb, 512], row (2gp+i)*128+d of the slice
        Wo_c = Wo[qs, :] * SWO
        woh_f, wol_f = _split_fp8(Wo_c, 1.0)
        def wopack(wo8):
            w = wo8.astype(np.float32).reshape(2, 2, 128, 4, 512)
            return np.ascontiguousarray(
                w.transpose(2, 0, 1, 3, 4)
            ).astype(NPF8)
        in_maps.append({
            "xh": xh_b[b],
            "xl": xl_b[b],
            "wqh": wqh, "wql": wql,
            "wkh": wkh, "wkl": wkl,
            "wvh": wvh, "wvl": wvl,
            "woh": wopack(woh_f), "wol": wopack(wol_f),
            "wsq": (Wqg[:, qs].sum(0) * SW).astype(NPBF)[None, :],
            "wsk": (Wkg[:, ks].sum(0) * SW).astype(NPBF)[None, :],
            "wsv": (Wvg[:, ks].sum(0) * SW).astype(NPBF)[None, :],
            "brow": brow[b].astype(NPBF)[None, :],
            "bqr": (bq_full[qs] * SX * SW).astype(NPBF)[None, :],
            "bkr": (bk_full[ks] * SX * SW).astype(NPBF)[None, :],
            "bvr": (bv_full[ks] * SX * SW).astype(NPBF)[None, :],
            "ones_row": np.ones((1, S), np.float32).astype(NPBF),
            "cos_t": cos_t.astype(NPBF),
            "sin_t": sin_t.astype(NPBF),
            "prot": prot.astype(NPBF),
            "onesc": np.ones((128, 128), np.float32).astype(NPBF),
        })
    return in_maps, has_bias


def kernel(x, ln_gamma, ln_beta, Wq, Wk, Wv, Wo):
    from concourse.bass_utils import run_bass_kernel_spmd

    in_maps, has_bias = make_in_maps(x, ln_gamma, ln_beta, Wq, Wk, Wv, Wo)
    nc = get_program(has_bias)
    res = run_bass_kernel_spmd(nc, in_maps, core_ids=list(range(8)))
    outs = [np.asarray(m["outp"], np.float32) for m in res.results]
    out = np.empty((B, S, H), np.float32)
    for b in range(B):
        out[b] = (outs[NKV * b] + outs[NKV * b + 1]) + (
            outs[NKV * b + 2] + outs[NKV * b + 3]
        )
    return out


# revision 33
# speedup vs baseline: 1.0137x; 1.0137x over previous
"""Trainium2 Bass kernel for pre-LN multi-head GQA attention (B=2, S=2048, H=2048,
NH=16, D=128, NKV=4, causal, RoPE).

Sharding: 8 cores = 2 batches x 4 KV groups. Core c handles batch c//4 and KV head
c%4 (its 4 query heads; Wq/Wk/Wv column-sharded by head, Wo row-sharded). Each core
computes a partial output [S, H] (bf16); the host sums the 4 per-batch partials.

v2 dataflow (per core, per 512-wide s-chunk):
  Host folds ln_gamma into the weights and the per-token rstd into the streamed
  activations (x' = rstd * x, transposed), then splits both x' and the weights
  into fp8e4m3 hi + unscaled-residual lo parts (one shared PSUM scale). QKV
  projections run as fp8 DoubleRow matmuls (256-deep contraction, 0.5 cyc/row),
  3 term streams: hi*hi, lo*hi, hi*lo. The LayerNorm mean correction is a bf16
  rank-1 matmul (wsum^T x brow, brow = -mu*rstd) in the same PSUM group.
  RoPE on Q^T/K^T in bf16 (PE rotation matmul + DVE/Pool elementwise); V^T is
  transposed to V via PE (bf16). Attention per head in the k-partition layout:
  logits^T = K^T.T Q^T (bf16 -> fp32 PSUM), exp on ACT -> e bf16, causal mask via
  affine_select on diagonal blocks, denominators as per-s-block column matmuls
  (lhsT = e block, rhs = ones column -> [128,1] PSUM cols, nearly free),
  ctx^T = V.T e (bf16). Reciprocal on tiny columns; broadcast back to rows via
  transpose + flatten-DMA + ones-column rank-1. ctx is evicted unnormalized
  (bf16), then normalized+quantized to fp8 hi/lo. Output projection is fp8
  DoubleRow over head pairs (3 terms), evicted to bf16 stages (ACT/DVE/Pool
  round-robin) and DMAd out.
"""

import sys

for p in ("/opt/trn_rl_repo",):
    if p not in sys.path:
        sys.path.append(p)

import numpy as np
import ml_dtypes

import concourse.bass as bass
import concourse.tile as tile
from concourse import bacc
from concourse import mybir
from concourse.masks import make_identity

F32 = mybir.dt.float32
BF16 = mybir.dt.bfloat16
FP8 = mybir.dt.float8e4
ALU = mybir.AluOpType
ACTF = mybir.ActivationFunctionType
DR = mybir.MatmulPerfMode.DoubleRow

NPF8 = ml_dtypes.float8_e4m3
NPBF = ml_dtypes.bfloat16

B, S, H = 2, 2048, 2048
NH, D, NKV = 16, 128, 4
G = NH // NKV  # query heads per KV head (= heads per core)
EPS = 1e-6
MIN_WIN, MAX_WIN = 1.0, 10000.0
SCALE = 1.0 / float(np.sqrt(np.float32(D)))
CHUNK = 512
NCH = S // CHUNK  # 4
NJP = H // 256  # 8 h-chunk pairs
SX = 2.0
SW = 64.0
SCTX = 8.0
SWO = 64.0
INV_PROJ = 1.0 / (SX * SW)
INV_OUT = 1.0 / (SCTX * SWO)


def build_program(has_bias: bool) -> bass.Bass:
    nc = bacc.Bacc(
        "TRN2",
        target_bir_lowering=False,
        debug=False,
        enable_asserts=False,
        num_devices=8,
    )
    xh_d = nc.dram_tensor("xh", [128, NJP, 2, S], FP8, kind="ExternalInput").ap()
    xl_d = nc.dram_tensor("xl", [128, NJP, 2, S], FP8, kind="ExternalInput").ap()
    wqh_d = nc.dram_tensor("wqh", [128, NJP, 2, G * D], FP8, kind="ExternalInput").ap()
    wql_d = nc.dram_tensor("wql", [128, NJP, 2, G * D], FP8, kind="ExternalInput").ap()
    wkh_d = nc.dram_tensor("wkh", [128, NJP, 2, D], FP8, kind="ExternalInput").ap()
    wkl_d = nc.dram_tensor("wkl", [128, NJP, 2, D], FP8, kind="ExternalInput").ap()
    wvh_d = nc.dram_tensor("wvh", [128, NJP, 2, D], FP8, kind="ExternalInput").ap()
    wvl_d = nc.dram_tensor("wvl", [128, NJP, 2, D], FP8, kind="ExternalInput").ap()
    woh_d = nc.dram_tensor("woh", [128, 2, 2, 4, 512], FP8, kind="ExternalInput").ap()
    wol_d = nc.dram_tensor("wol", [128, 2, 2, 4, 512], FP8, kind="ExternalInput").ap()
    wsq_d = nc.dram_tensor("wsq", [1, G * D], BF16, kind="ExternalInput").ap()
    wsk_d = nc.dram_tensor("wsk", [1, D], BF16, kind="ExternalInput").ap()
    wsv_d = nc.dram_tensor("wsv", [1, D], BF16, kind="ExternalInput").ap()
    brow_d = nc.dram_tensor("brow", [1, S], BF16, kind="ExternalInput").ap()
    bq_d = nc.dram_tensor("bqr", [1, G * D], BF16, kind="ExternalInput").ap()
    bk_d = nc.dram_tensor("bkr", [1, D], BF16, kind="ExternalInput").ap()
    bv_d = nc.dram_tensor("bvr", [1, D], BF16, kind="ExternalInput").ap()
    ones_row_d = nc.dram_tensor("ones_row", [1, S], BF16, kind="ExternalInput").ap()
    cos_d = nc.dram_tensor("cos_t", [128, S], BF16, kind="ExternalInput").ap()
    sin_d = nc.dram_tensor("sin_t", [128, S], BF16, kind="ExternalInput").ap()
    prot_d = nc.dram_tensor("prot", [128, 128], BF16, kind="ExternalInput").ap()
    ones_d = nc.dram_tensor("onesc", [128, 128], BF16, kind="ExternalInput").ap()
    outp = nc.dram_tensor("outp", [S, H], BF16, kind="ExternalOutput").ap()

    with tile.TileContext(nc) as tc:
        with (
            tc.tile_pool(name="singles", bufs=1) as singles,
            tc.tile_pool(name="xp", bufs=2) as xp,
            tc.tile_pool(name="work", bufs=4) as work,
            tc.tile_pool(name="qp", bufs=6) as qp,
            tc.tile_pool(name="ep", bufs=6) as ep,
            tc.tile_pool(name="cp", bufs=2) as cp,
            tc.tile_pool(name="cup", bufs=5) as cup,
            tc.tile_pool(name="stg", bufs=6) as stg,
            # PSUM budget (16KB/partition): psL 3x2KB + psA 2x2KB + psC 1x2KB
            #  + psD 1x2KB(64B used) + psM 1x2KB = 16KB
            tc.tile_pool(name="psL", bufs=3, space="PSUM") as psL,
            tc.tile_pool(name="psA", bufs=2, space="PSUM") as psA,
            tc.tile_pool(name="psC", bufs=1, space="PSUM") as psC,
            tc.tile_pool(name="psD", bufs=1, space="PSUM") as psD,
            tc.tile_pool(name="psM", bufs=1, space="PSUM") as psM,
        ):
            # ---- resident constants/weights ----
            ones_sb = singles.tile([128, 128], BF16)
            nc.scalar.dma_start(ones_sb, ones_d)
            prot_sb = singles.tile([128, 128], BF16)
            nc.scalar.dma_start(prot_sb, prot_d)
            wsq_sb = singles.tile([1, G * D], BF16)
            nc.scalar.dma_start(wsq_sb, wsq_d)
            wsk_sb = singles.tile([1, D], BF16)
            nc.scalar.dma_start(wsk_sb, wsk_d)
            wsv_sb = singles.tile([1, D], BF16)
            nc.scalar.dma_start(wsv_sb, wsv_d)
            brow_sb = singles.tile([1, S], BF16)
            nc.scalar.dma_start(brow_sb, brow_d)
            wkh_sb = singles.tile([128, NJP, 2, D], FP8)
            nc.gpsimd.dma_start(wkh_sb, wkh_d)
            wvh_sb = singles.tile([128, NJP, 2, D], FP8)
            nc.scalar.dma_start(wvh_sb, wvh_d)
            wkl_sb = singles.tile([128, NJP, 2, D], FP8)
            nc.scalar.dma_start(wkl_sb, wkl_d)
            wvl_sb = singles.tile([128, NJP, 2, D], FP8)
            nc.scalar.dma_start(wvl_sb, wvl_d)
            wqh_sb = singles.tile([128, NJP, 2, G * D], FP8)
            nc.gpsimd.dma_start(wqh_sb[:, 0:4], wqh_d[:, 0:4])
            nc.gpsimd.dma_start(wqh_sb[:, 4:8], wqh_d[:, 4:8])
            wql_sb = singles.tile([128, NJP, 2, G * D], FP8)
            nc.scalar.dma_start(wql_sb[:, 0:4], wql_d[:, 0:4])
            nc.scalar.dma_start(wql_sb[:, 4:8], wql_d[:, 4:8])
            cos_sb = singles.tile([128, S], BF16)
            nc.gpsimd.dma_start(cos_sb, cos_d)
            sin_sb = singles.tile([128, S], BF16)
            nc.gpsimd.dma_start(sin_sb, sin_d)
            woh_sb = singles.tile([128, 2, 2, 4, 512], FP8)
            nc.gpsimd.dma_start(woh_sb, woh_d)
            wol_sb = singles.tile([128, 2, 2, 4, 512], FP8)
            nc.gpsimd.dma_start(wol_sb, wol_d)
            if has_bias:
                bq_sb = singles.tile([1, G * D], BF16)
                nc.scalar.dma_start(bq_sb, bq_d)
                bk_sb = singles.tile([1, D], BF16)
                nc.scalar.dma_start(bk_sb, bk_d)
                bv_sb = singles.tile([1, D], BF16)
                nc.scalar.dma_start(bv_sb, bv_d)
                onesr_sb = singles.tile([1, S], BF16)
                nc.scalar.dma_start(onesr_sb, ones_row_d)
            identf = singles.tile([128, 128], F32)
            make_identity(nc, identf)
            identb = singles.tile([128, 128], BF16)
            nc.vector.tensor_copy(identb, identf)
            kT_sb = singles.tile([128, S], BF16)  # roped K^T
            v_sb = singles.tile([128, S // 128, D], BF16)  # V natural

            def proj_group(pt, wh, wl, xh, xl, ws, bias, sl):
                """Accumulate one projection tile [128, CHUNK] into psum pt."""
                n = 0
                last = 3 * NJP + 1 + (1 if bias is not None else 0)
                for term in range(3):
                    w_, x_ = ((wh, xh), (wh, xl), (wl, xh))[term]
                    for j in range(NJP):
                        n += 1
                        nc.tensor.matmul(
                            pt,
                            w_[:, j],
                            x_[:, j],
                            start=(n == 1),
                            stop=False,
                            perf_mode=DR,
                            skip_group_check=(n > 1),
                        )
                n += 1
                nc.tensor.matmul(
                    pt, ws, brow_sb[:, sl], start=False, stop=(n == last),
                    skip_group_check=(n != last),
                )
                if bias is not None:
                    nc.tensor.matmul(
                        pt, bias, onesr_sb[:, sl], start=False, stop=True,
                    )

            def rope(out, raw, cos_c, sin_c):
                """out(bf16) = raw*cos + (P_rot@raw)*sin for one [128, CHUNK]."""
                rps = psL.tile([128, CHUNK], F32, tag="pl", name="rot")
                nc.tensor.matmul(rps, prot_sb, raw, start=True, stop=True)
                tmp = work.tile([128, CHUNK], BF16, tag="ropetmp")
                nc.vector.tensor_mul(tmp, rps, sin_c)
                rc = work.tile([128, CHUNK], BF16, tag="ropecos")
                nc.gpsimd.tensor_mul(rc, raw, cos_c)
                nc.vector.tensor_add(out, rc, tmp)

            def emit_proj(qb):
                """QKV projections + rope for chunk qb; returns q tiles."""
                sl = slice(qb * CHUNK, (qb + 1) * CHUNK)
                cos_c = cos_sb[:, sl]
                sin_c = sin_sb[:, sl]

                xh_sb = xp.tile([128, NJP, 2, CHUNK], FP8, tag="xh")
                for jq in range(4):
                    nc.sync.dma_start(xh_sb[:, 2 * jq:2 * jq + 2],
                                      xh_d[:, 2 * jq:2 * jq + 2, :, sl])
                xl_sb = xp.tile([128, NJP, 2, CHUNK], FP8, tag="xl")
                for jq in range(4):
                    nc.sync.dma_start(xl_sb[:, 2 * jq:2 * jq + 2],
                                      xl_d[:, 2 * jq:2 * jq + 2, :, sl])

                pk = psA.tile([128, CHUNK], F32, tag="proj", name="pk")
                proj_group(pk, wkh_sb, wkl_sb, xh_sb, xl_sb, wsk_sb,
                           bk_sb if has_bias else None, sl)
                pv = psA.tile([128, CHUNK], F32, tag="proj", name="pv")
                proj_group(pv, wvh_sb, wvl_sb, xh_sb, xl_sb, wsv_sb,
                           bv_sb if has_bias else None, sl)

                kraw = work.tile([128, CHUNK], BF16, tag="kraw")
                nc.scalar.mul(kraw, pk, INV_PROJ)
                rope(kT_sb[:, sl], kraw, cos_c, sin_c)

                vt = work.tile([128, CHUNK], BF16, tag="vt")
                nc.scalar.mul(vt, pv, INV_PROJ)
                ptv = psL.tile([128, 4, 128], BF16, tag="pl", name="vtr")
                for m in range(4):
                    nc.tensor.transpose(
                        ptv[:, m, :], vt[:, m * 128:(m + 1) * 128], identb
                    )
                nc.vector.tensor_scalar_mul(
                    v_sb[:, qb * 4: qb * 4 + 4, :], ptv, 1.0
                )

                qts = []
                for g_ in range(G):
                    cs = slice(g_ * D, (g_ + 1) * D)
                    pq = psA.tile([128, CHUNK], F32, tag="proj", name=f"pq{g_}")
                    proj_group(pq, wqh_sb[:, :, :, cs], wql_sb[:, :, :, cs],
                               xh_sb, xl_sb, wsq_sb[:, cs],
                               bq_sb[:, cs] if has_bias else None, sl)
                    qraw = work.tile([128, CHUNK], BF16, tag="qraw")
                    nc.scalar.mul(qraw, pq, INV_PROJ)
                    q_g = qp.tile([128, CHUNK], BF16, tag="q")
                    rope(q_g, qraw, cos_c, sin_c)
                    qts.append(q_g)
                return qts

            def emit_attention(qb, qts):
                kmax = 4 * (qb + 1)
                recs = []
                ctxs_u = []
                for g_ in range(G):
                    pctx = psC.tile([128, CHUNK], F32, tag="ctx")
                    den_ps = psD.tile([128, 4], F32, tag="den")
                    nden = 0
                    tden = sum(4 - max(0, kb - 4 * qb) for kb in range(kmax))
                    for kb in range(kmax):
                        i_d = max(0, kb - 4 * qb)  # first valid s-block
                        vs = slice(i_d * 128, CHUNK)
                        pl = psL.tile([128, CHUNK], F32, tag="pl")
                        nc.tensor.matmul(
                            pl[:, vs],
                            kT_sb[:, kb * 128:(kb + 1) * 128],
                            qts[g_][:, vs],
                            start=True,
                            stop=True,
                        )
                        e = ep.tile([128, CHUNK], BF16, tag="e")
                        nc.scalar.activation(e[:, vs], pl[:, vs], ACTF.Exp,
                                             scale=SCALE)
                        if kb >= 4 * qb:
                            # causal triangle within the diagonal 128-block
                            ds = slice(i_d * 128, (i_d + 1) * 128)
                            nc.gpsimd.affine_select(
                                out=e[:, ds],
                                in_=e[:, ds],
                                compare_op=ALU.is_ge,
                                fill=0.0,
                                base=0,
                                pattern=[[1, 128]],
                                channel_multiplier=-1,
                            )
                        for m in range(i_d, 4):
                            nden += 1
                            nc.tensor.matmul(
                                den_ps[:, m:m + 1],
                                e[:, m * 128:(m + 1) * 128],
                                ones_sb[:, 0:1],
                                start=(nden == 1),
                                stop=(nden == tden),
                                skip_group_check=(1 < nden < tden),
                            )
                        nc.tensor.matmul(
                            pctx[:, vs],
                            v_sb[:, kb, :],
                            e[:, vs],
                            start=(kb == 0),
                            stop=(kb == kmax - 1),
                        )
                    # reciprocal chain: cols -> row -> flatten
                    rcol = work.tile([128, 4], F32, tag="rcol")
                    nc.vector.reciprocal(rcol, den_ps)
                    rcolb = work.tile([128, 4], BF16, tag="rcolb")
                    nc.scalar.copy(rcolb, rcol)
                    prt = psL.tile([4, 128], BF16, tag="pl", name="rect")
                    nc.tensor.transpose(prt, rcolb, identb)
                    rrow4 = work.tile([4, 128], BF16, tag="rrow4")
                    nc.scalar.copy(rrow4, prt)
                    rflat = work.tile([1, 4, 128], BF16, tag="rflat")
                    nc.sync.dma_start(rflat, rrow4)
                    recs.append(rflat)
                    cu = cup.tile([128, CHUNK], BF16, tag="cu")
                    nc.vector.tensor_scalar_mul(cu, pctx, 1.0)
                    ctxs_u.append(cu)

                # normalize + quantize ctx -> fp8 hi/lo
                ctx_hi = cp.tile([128, G, CHUNK], FP8, tag="chi")
                ctx_lo = cp.tile([128, G, CHUNK], FP8, tag="clo")
                for g_ in range(G):
                    prb = psL.tile([128, CHUNK], F32, tag="pl", name="prb")
                    nc.tensor.matmul(prb, ones_sb[0:1, :], recs[g_][0:1],
                                     start=True, stop=True)
                    cbf = cp.tile([128, CHUNK], BF16, tag="cbf")
                    nc.vector.scalar_tensor_tensor(
                        out=cbf, in0=ctxs_u[g_], scalar=SCTX, in1=prb,
                        op0=ALU.mult, op1=ALU.mult,
                    )
                    nc.scalar.copy(ctx_hi[:, g_, :], cbf)
                    nc.gpsimd.tensor_sub(ctx_lo[:, g_, :], cbf,
                                         ctx_hi[:, g_, :])
                return ctx_hi, ctx_lo

            def emit_outproj(qb, ctx_hi, ctx_lo):
                for sm in range(4):
                    row = slice(qb * CHUNK + sm * 128, qb * CHUNK + (sm + 1) * 128)
                    ss = slice(sm * 128, (sm + 1) * 128)
                    for cb in range(4):
                        if qb == NCH - 1 and (sm * 4 + cb) % 2 == 1:
                            po = psA.tile([128, CHUNK], F32, tag="proj",
                                          name="po")
                        else:
                            po = psM.tile([128, 512], F32, tag="po", name="po")
                        n = 0  # 6 DR terms
                        for term in range(3):
                            ch, wo_ = ((ctx_hi, woh_sb), (ctx_lo, woh_sb),
                                       (ctx_hi, wol_sb))[term]
                            for gp in range(2):
                                n += 1
                                nc.tensor.matmul(
                                    po,
                                    ch[:, 2 * gp:2 * gp + 2, ss],
                                    wo_[:, gp, :, cb, :],
                                    start=(n == 1),
                                    stop=(n == 6),
                                    perf_mode=DR,
                                    skip_group_check=(1 < n < 6),
                                )  # gp-major
                        stage = stg.tile([128, 512], BF16, tag="stage")
                        nc.vector.tensor_scalar_mul(stage, po, INV_OUT)
                        nc.sync.dma_start(
                            outp[row, cb * 512:(cb + 1) * 512], stage
                        )

            # software-pipelined chunk loop: attention(qb) emits before
            # proj(qb+1), and outproj(qb) after it, so next-chunk projection
            # matmuls fill attention's exp-latency gaps and outproj fills
            # the following chunk's.
            qts = emit_proj(0)
            for qb in range(NCH):
                ctx_hi, ctx_lo = emit_attention(qb, qts)
                if qb + 1 < NCH:
                    qts = emit_proj(qb + 1)
                emit_outproj(qb, ctx_hi, ctx_lo)
    nc.compile()
    return nc


_PROGRAMS: dict[bool, bass.Bass] = {}


def get_program(has_bias: bool) -> bass.Bass:
    if has_bias not in _PROGRAMS:
        _PROGRAMS[has_bias] = build_program(has_bias)
    return _PROGRAMS[has_bias]


def _split_fp8(t, scale):
    """Return (hi, lo) fp8e4m3 arrays: t*scale = hi + lo (unscaled residual)."""
    ts = np.asarray(t, np.float32) * scale
    assert np.abs(ts).max() < 239.0, f"fp8 overflow {np.abs(ts).max()}"
    hi = ts.astype(NPF8)
    lo = (ts - hi.astype(np.float32)).astype(NPF8)
    return hi, lo


def _pack_pairs(w):
    """[H, M] -> [128, NJP, 2, M]: row (2j+i)*128+p -> [p, j, i, :]."""
    Hh, M = w.shape
    return np.ascontiguousarray(
        w.reshape(NJP, 2, 128, M).transpose(2, 0, 1, 3)
    )


def make_in_maps(x, ln_gamma, ln_beta, Wq, Wk, Wv, Wo):
    x = np.asarray(x, np.float64)
    g = np.asarray(ln_gamma, np.float64)
    be = np.asarray(ln_beta, np.float64)
    Wq = np.asarray(Wq, np.float64)
    Wk = np.asarray(Wk, np.float64)
    Wv = np.asarray(Wv, np.float64)
    Wo = np.asarray(Wo, np.float64)

    Wqg = Wq * g[:, None]
    Wkg = Wk * g[:, None]
    Wvg = Wv * g[:, None]
    bq_full = be @ Wq
    bk_full = be @ Wk
    bv_full = be @ Wv
    has_bias = bool(np.any(be != 0.0))

    # host LN stats
    mu = x.mean(-1, keepdims=True)  # [B, S, 1]
    var = ((x - mu) ** 2).mean(-1, keepdims=True)
    rstd = 1.0 / np.sqrt(var + EPS)
    xs = x * rstd  # x' = rstd * x
    brow = (-mu[..., 0] * rstd[..., 0]) * SX  # [B, S]

    # rope tables (halves duplicated)
    half = D // 2
    ts = MIN_WIN * (MAX_WIN / MIN_WIN) ** (
        2.0 * np.arange(half, dtype=np.float64) / D
    )
    ang = np.arange(S, dtype=np.float64)[None, :] / ts[:, None]
    cos_t = np.concatenate([np.cos(ang), np.cos(ang)], axis=0)
    sin_t = np.concatenate([np.sin(ang), np.sin(ang)], axis=0)

    prot = np.zeros((128, 128), np.float32)
    for m in range(half):
        prot[m + half, m] = -1.0
        prot[m, m + half] = 1.0

    # per-batch x' streams: [128, NJP, 2, S]
    xh_b, xl_b = [], []
    for b in range(B):
        xT = np.ascontiguousarray(xs[b].T)  # [H, S]
        hi, lo = _split_fp8(xT, SX)
        xh_b.append(_pack_pairs(hi.astype(np.float32)).astype(NPF8))
        xl_b.append(_pack_pairs(lo.astype(np.float32)).astype(NPF8))

    def wpack(Wg):
        hi, lo = _split_fp8(Wg, SW)
        return (_pack_pairs(hi.astype(np.float32)).astype(NPF8),
                _pack_pairs(lo.astype(np.float32)).astype(NPF8))

    in_maps = []
    for c in range(8):
        b, h = divmod(c, NKV)
        qs = slice(h * G * D, (h + 1) * G * D)
        ks = slice(h * D, (h + 1) * D)
        wqh, wql = wpack(Wqg[:, qs])
        wkh, wkl = wpack(Wkg[:, ks])
        wvh, wvl = wpack(Wvg[:, ks])
        # Wo rows for this core's 4 heads: [G*D, H]; head-pair packed:
        # [128(d), gp, i, cb, 512], row (2gp+i)*128+d of the slice
        Wo_c = Wo[qs, :] * SWO
        woh_f, wol_f = _split_fp8(Wo_c, 1.0)
        def wopack(wo8):
            w = wo8.astype(np.float32).reshape(2, 2, 128, 4, 512)
            return np.ascontiguousarray(
                w.transpose(2, 0, 1, 3, 4)
            ).astype(NPF8)
        in_maps.append({
            "xh": xh_b[b],
            "xl": xl_b[b],
            "wqh": wqh, "wql": wql,
            "wkh": wkh, "wkl": wkl,
            "wvh": wvh, "wvl": wvl,
            "woh": wopack(woh_f), "wol": wopack(wol_f),
            "wsq": (Wqg[:, qs].sum(0) * SW).astype(NPBF)[None, :],
            "wsk": (Wkg[:, ks].sum(0) * SW).astype(NPBF)[None, :],
            "wsv": (Wvg[:, ks].sum(0) * SW).astype(NPBF)[None, :],
            "brow": brow[b].astype(NPBF)[None, :],
            "bqr": (bq_full[qs] * SX * SW).astype(NPBF)[None, :],
            "bkr": (bk_full[ks] * SX * SW).astype(NPBF)[None, :],
            "bvr": (bv_full[ks] * SX * SW).astype(NPBF)[None, :],
            "ones_row": np.ones((1, S), np.float32).astype(NPBF),
            "cos_t": cos_t.astype(NPBF),
            "sin_t": sin_t.astype(NPBF),
            "prot": prot.astype(NPBF),
            "onesc": np.ones((128, 128), np.float32).astype(NPBF),
        })
    return in_maps, has_bias


def kernel(x, ln_gamma, ln_beta, Wq, Wk, Wv, Wo):
    from concourse.bass_utils import run_bass_kernel_spmd

    in_maps, has_bias = make_in_maps(x, ln_gamma, ln_beta, Wq, Wk, Wv, Wo)
    nc = get_program(has_bias)
    res = run_bass_kernel_spmd(nc, in_maps, core_ids=list(range(8)))
    outs = [np.asarray(m["outp"], np.float32) for m in res.results]
    out = np.empty((B, S, H), np.float32)
    for b in range(B):
        out[b] = (outs[NKV * b] + outs[NKV * b + 1]) + (
            outs[NKV * b + 2] + outs[NKV * b + 3]
        )
    return out


# revision 34
# speedup vs baseline: 1.0270x; 1.0131x over previous
"""Trainium2 Bass kernel for pre-LN multi-head GQA attention (B=2, S=2048, H=2048,
NH=16, D=128, NKV=4, causal, RoPE).

Sharding: 8 cores = 2 batches x 4 KV groups. Core c handles batch c//4 and KV head
c%4 (its 4 query heads; Wq/Wk/Wv column-sharded by head, Wo row-sharded). Each core
computes a partial output [S, H] (bf16); the host sums the 4 per-batch partials.

v2 dataflow (per core, per 512-wide s-chunk):
  Host folds ln_gamma into the weights and the per-token rstd into the streamed
  activations (x' = rstd * x, transposed), then splits both x' and the weights
  into fp8e4m3 hi + unscaled-residual lo parts (one shared PSUM scale). QKV
  projections run as fp8 DoubleRow matmuls (256-deep contraction, 0.5 cyc/row),
  3 term streams: hi*hi, lo*hi, hi*lo. The LayerNorm mean correction is a bf16
  rank-1 matmul (wsum^T x brow, brow = -mu*rstd) in the same PSUM group.
  RoPE on Q^T/K^T in bf16 (PE rotation matmul + DVE/Pool elementwise); V^T is
  transposed to V via PE (bf16). Attention per head in the k-partition layout:
  logits^T = K^T.T Q^T (bf16 -> fp32 PSUM), exp on ACT -> e bf16, causal mask via
  affine_select on diagonal blocks, denominators as per-s-block column matmuls
  (lhsT = e block, rhs = ones column -> [128,1] PSUM cols, nearly free),
  ctx^T = V.T e (bf16). Reciprocal on tiny columns; broadcast back to rows via
  transpose + flatten-DMA + ones-column rank-1. ctx is evicted unnormalized
  (bf16), then normalized+quantized to fp8 hi/lo. Output projection is fp8
  DoubleRow over head pairs (3 terms), evicted to bf16 stages (ACT/DVE/Pool
  round-robin) and DMAd out.
"""

import sys

for p in ("/opt/trn_rl_repo",):
    if p not in sys.path:
        sys.path.append(p)

import numpy as np
import ml_dtypes

import concourse.bass as bass
import concourse.tile as tile
from concourse import bacc
from concourse import mybir
from concourse.masks import make_identity

F32 = mybir.dt.float32
BF16 = mybir.dt.bfloat16
FP8 = mybir.dt.float8e4
ALU = mybir.AluOpType
ACTF = mybir.ActivationFunctionType
DR = mybir.MatmulPerfMode.DoubleRow

NPF8 = ml_dtypes.float8_e4m3
NPBF = ml_dtypes.bfloat16

B, S, H = 2, 2048, 2048
NH, D, NKV = 16, 128, 4
G = NH // NKV  # query heads per KV head (= heads per core)
EPS = 1e-6
MIN_WIN, MAX_WIN = 1.0, 10000.0
SCALE = 1.0 / float(np.sqrt(np.float32(D)))
CHUNK = 512
NCH = S // CHUNK  # 4
NJP = H // 256  # 8 h-chunk pairs
SX = 2.0
SW = 64.0
SCTX = 8.0
SWO = 64.0
INV_PROJ = 1.0 / (SX * SW)
INV_OUT = 1.0 / (SCTX * SWO)


def build_program(has_bias: bool) -> bass.Bass:
    nc = bacc.Bacc(
        "TRN2",
        target_bir_lowering=False,
        debug=False,
        enable_asserts=False,
        num_devices=8,
    )
    xh_d = nc.dram_tensor("xh", [128, NJP, 2, S], FP8, kind="ExternalInput").ap()
    xl_d = nc.dram_tensor("xl", [128, NJP, 2, S], FP8, kind="ExternalInput").ap()
    wqh_d = nc.dram_tensor("wqh", [128, NJP, 2, G * D], FP8, kind="ExternalInput").ap()
    wql_d = nc.dram_tensor("wql", [128, NJP, 2, G * D], FP8, kind="ExternalInput").ap()
    wkh_d = nc.dram_tensor("wkh", [128, NJP, 2, D], FP8, kind="ExternalInput").ap()
    wkl_d = nc.dram_tensor("wkl", [128, NJP, 2, D], FP8, kind="ExternalInput").ap()
    wvh_d = nc.dram_tensor("wvh", [128, NJP, 2, D], FP8, kind="ExternalInput").ap()
    wvl_d = nc.dram_tensor("wvl", [128, NJP, 2, D], FP8, kind="ExternalInput").ap()
    woh_d = nc.dram_tensor("woh", [128, 2, 2, 4, 512], FP8, kind="ExternalInput").ap()
    wol_d = nc.dram_tensor("wol", [128, 2, 2, 4, 512], FP8, kind="ExternalInput").ap()
    wsq_d = nc.dram_tensor("wsq", [1, G * D], BF16, kind="ExternalInput").ap()
    wsk_d = nc.dram_tensor("wsk", [1, D], BF16, kind="ExternalInput").ap()
    wsv_d = nc.dram_tensor("wsv", [1, D], BF16, kind="ExternalInput").ap()
    brow_d = nc.dram_tensor("brow", [1, S], BF16, kind="ExternalInput").ap()
    bq_d = nc.dram_tensor("bqr", [1, G * D], BF16, kind="ExternalInput").ap()
    bk_d = nc.dram_tensor("bkr", [1, D], BF16, kind="ExternalInput").ap()
    bv_d = nc.dram_tensor("bvr", [1, D], BF16, kind="ExternalInput").ap()
    ones_row_d = nc.dram_tensor("ones_row", [1, S], BF16, kind="ExternalInput").ap()
    cos_d = nc.dram_tensor("cos_t", [128, S], BF16, kind="ExternalInput").ap()
    sin_d = nc.dram_tensor("sin_t", [128, S], BF16, kind="ExternalInput").ap()
    prot_d = nc.dram_tensor("prot", [128, 128], BF16, kind="ExternalInput").ap()
    ones_d = nc.dram_tensor("onesc", [128, 128], BF16, kind="ExternalInput").ap()
    outp = nc.dram_tensor("outp", [S, H], BF16, kind="ExternalOutput").ap()

    with tile.TileContext(nc) as tc:
        with (
            tc.tile_pool(name="singles", bufs=1) as singles,
            tc.tile_pool(name="xp", bufs=2) as xp,
            tc.tile_pool(name="work", bufs=4) as work,
            tc.tile_pool(name="qp", bufs=6) as qp,
            tc.tile_pool(name="ep", bufs=6) as ep,
            tc.tile_pool(name="cp", bufs=2) as cp,
            tc.tile_pool(name="cup", bufs=5) as cup,
            tc.tile_pool(name="stg", bufs=6) as stg,
            # PSUM budget (16KB/partition): psL 3x2KB + psA 2x2KB + psC 1x2KB
            #  + psD 1x2KB(64B used) + psM 1x2KB = 16KB
            tc.tile_pool(name="psL", bufs=3, space="PSUM") as psL,
            tc.tile_pool(name="psA", bufs=2, space="PSUM") as psA,
            tc.tile_pool(name="psC", bufs=1, space="PSUM") as psC,
            tc.tile_pool(name="psD", bufs=1, space="PSUM") as psD,
            tc.tile_pool(name="psM", bufs=1, space="PSUM") as psM,
        ):
            # ---- resident constants/weights ----
            ones_sb = singles.tile([128, 128], BF16)
            nc.scalar.dma_start(ones_sb, ones_d)
            prot_sb = singles.tile([128, 128], BF16)
            nc.scalar.dma_start(prot_sb, prot_d)
            wsq_sb = singles.tile([1, G * D], BF16)
            nc.scalar.dma_start(wsq_sb, wsq_d)
            wsk_sb = singles.tile([1, D], BF16)
            nc.scalar.dma_start(wsk_sb, wsk_d)
            wsv_sb = singles.tile([1, D], BF16)
            nc.scalar.dma_start(wsv_sb, wsv_d)
            brow_sb = singles.tile([1, S], BF16)
            nc.scalar.dma_start(brow_sb, brow_d)
            wkh_sb = singles.tile([128, NJP, 2, D], FP8)
            nc.gpsimd.dma_start(wkh_sb, wkh_d)
            wvh_sb = singles.tile([128, NJP, 2, D], FP8)
            nc.gpsimd.dma_start(wvh_sb, wvh_d)
            wkl_sb = singles.tile([128, NJP, 2, D], FP8)
            nc.gpsimd.dma_start(wkl_sb, wkl_d)
            wvl_sb = singles.tile([128, NJP, 2, D], FP8)
            nc.gpsimd.dma_start(wvl_sb, wvl_d)
            wqh_sb = singles.tile([128, NJP, 2, G * D], FP8)
            nc.gpsimd.dma_start(wqh_sb[:, 0:4], wqh_d[:, 0:4])
            nc.gpsimd.dma_start(wqh_sb[:, 4:8], wqh_d[:, 4:8])
            wql_sb = singles.tile([128, NJP, 2, G * D], FP8)
            nc.gpsimd.dma_start(wql_sb[:, 0:4], wql_d[:, 0:4])
            nc.gpsimd.dma_start(wql_sb[:, 4:8], wql_d[:, 4:8])
            cos_sb = singles.tile([128, S], BF16)
            nc.gpsimd.dma_start(cos_sb, cos_d)
            sin_sb = singles.tile([128, S], BF16)
            nc.gpsimd.dma_start(sin_sb, sin_d)
            woh_sb = singles.tile([128, 2, 2, 4, 512], FP8)
            nc.gpsimd.dma_start(woh_sb, woh_d)
            wol_sb = singles.tile([128, 2, 2, 4, 512], FP8)
            nc.gpsimd.dma_start(wol_sb, wol_d)
            if has_bias:
                bq_sb = singles.tile([1, G * D], BF16)
                nc.scalar.dma_start(bq_sb, bq_d)
                bk_sb = singles.tile([1, D], BF16)
                nc.scalar.dma_start(bk_sb, bk_d)
                bv_sb = singles.tile([1, D], BF16)
                nc.scalar.dma_start(bv_sb, bv_d)
                onesr_sb = singles.tile([1, S], BF16)
                nc.scalar.dma_start(onesr_sb, ones_row_d)
            identf = singles.tile([128, 128], F32)
            make_identity(nc, identf)
            identb = singles.tile([128, 128], BF16)
            nc.vector.tensor_copy(identb, identf)
            kT_sb = singles.tile([128, S], BF16)  # roped K^T
            v_sb = singles.tile([128, S // 128, D], BF16)  # V natural

            def proj_group(pt, wh, wl, xh, xl, ws, bias, sl):
                """Accumulate one projection tile [128, CHUNK] into psum pt."""
                n = 0
                last = 3 * NJP + 1 + (1 if bias is not None else 0)
                for term in range(3):
                    w_, x_ = ((wh, xh), (wh, xl), (wl, xh))[term]
                    for j in range(NJP):
                        n += 1
                        nc.tensor.matmul(
                            pt,
                            w_[:, j],
                            x_[:, j],
                            start=(n == 1),
                            stop=False,
                            perf_mode=DR,
                            skip_group_check=(n > 1),
                        )
                n += 1
                nc.tensor.matmul(
                    pt, ws, brow_sb[:, sl], start=False, stop=(n == last),
                    skip_group_check=(n != last),
                )
                if bias is not None:
                    nc.tensor.matmul(
                        pt, bias, onesr_sb[:, sl], start=False, stop=True,
                    )

            def rope(out, raw, cos_c, sin_c):
                """out(bf16) = raw*cos + (P_rot@raw)*sin for one [128, CHUNK]."""
                rps = psL.tile([128, CHUNK], F32, tag="pl", name="rot")
                nc.tensor.matmul(rps, prot_sb, raw, start=True, stop=True)
                tmp = work.tile([128, CHUNK], BF16, tag="ropetmp")
                nc.vector.tensor_mul(tmp, rps, sin_c)
                rc = work.tile([128, CHUNK], BF16, tag="ropecos")
                nc.gpsimd.tensor_mul(rc, raw, cos_c)
                nc.vector.tensor_add(out, rc, tmp)

            def emit_proj(qb):
                """QKV projections + rope for chunk qb; returns q tiles."""
                sl = slice(qb * CHUNK, (qb + 1) * CHUNK)
                cos_c = cos_sb[:, sl]
                sin_c = sin_sb[:, sl]

                xh_sb = xp.tile([128, NJP, 2, CHUNK], FP8, tag="xh")
                for jq in range(4):
                    nc.sync.dma_start(xh_sb[:, 2 * jq:2 * jq + 2],
                                      xh_d[:, 2 * jq:2 * jq + 2, :, sl])
                xl_sb = xp.tile([128, NJP, 2, CHUNK], FP8, tag="xl")
                for jq in range(4):
                    nc.sync.dma_start(xl_sb[:, 2 * jq:2 * jq + 2],
                                      xl_d[:, 2 * jq:2 * jq + 2, :, sl])

                pk = psA.tile([128, CHUNK], F32, tag="proj", name="pk")
                proj_group(pk, wkh_sb, wkl_sb, xh_sb, xl_sb, wsk_sb,
                           bk_sb if has_bias else None, sl)
                pv = psA.tile([128, CHUNK], F32, tag="proj", name="pv")
                proj_group(pv, wvh_sb, wvl_sb, xh_sb, xl_sb, wsv_sb,
                           bv_sb if has_bias else None, sl)

                kraw = work.tile([128, CHUNK], BF16, tag="kraw")
                nc.scalar.mul(kraw, pk, INV_PROJ)
                rope(kT_sb[:, sl], kraw, cos_c, sin_c)

                vt = work.tile([128, CHUNK], BF16, tag="vt")
                nc.scalar.mul(vt, pv, INV_PROJ)
                ptv = psL.tile([128, 4, 128], BF16, tag="pl", name="vtr")
                for m in range(4):
                    nc.tensor.transpose(
                        ptv[:, m, :], vt[:, m * 128:(m + 1) * 128], identb
                    )
                nc.vector.tensor_scalar_mul(
                    v_sb[:, qb * 4: qb * 4 + 4, :], ptv, 1.0
                )

                qts = []
                for g_ in range(G):
                    cs = slice(g_ * D, (g_ + 1) * D)
                    pq = psA.tile([128, CHUNK], F32, tag="proj", name=f"pq{g_}")
                    proj_group(pq, wqh_sb[:, :, :, cs], wql_sb[:, :, :, cs],
                               xh_sb, xl_sb, wsq_sb[:, cs],
                               bq_sb[:, cs] if has_bias else None, sl)
                    qraw = work.tile([128, CHUNK], BF16, tag="qraw")
                    nc.scalar.mul(qraw, pq, INV_PROJ)
                    q_g = qp.tile([128, CHUNK], BF16, tag="q")
                    rope(q_g, qraw, cos_c, sin_c)
                    qts.append(q_g)
                return qts

            def emit_attention(qb, qts):
                kmax = 4 * (qb + 1)
                recs = []
                ctxs_u = []
                for g_ in range(G):
                    pctx = psC.tile([128, CHUNK], F32, tag="ctx")
                    den_ps = psD.tile([128, 4], F32, tag="den")
                    nden = 0
                    tden = sum(4 - max(0, kb - 4 * qb) for kb in range(kmax))
                    for kb in range(kmax):
                        i_d = max(0, kb - 4 * qb)  # first valid s-block
                        vs = slice(i_d * 128, CHUNK)
                        pl = psL.tile([128, CHUNK], F32, tag="pl")
                        nc.tensor.matmul(
                            pl[:, vs],
                            kT_sb[:, kb * 128:(kb + 1) * 128],
                            qts[g_][:, vs],
                            start=True,
                            stop=True,
                        )
                        e = ep.tile([128, CHUNK], BF16, tag="e")
                        nc.scalar.activation(e[:, vs], pl[:, vs], ACTF.Exp,
                                             scale=SCALE)
                        if kb >= 4 * qb:
                            # causal triangle within the diagonal 128-block
                            ds = slice(i_d * 128, (i_d + 1) * 128)
                            nc.gpsimd.affine_select(
                                out=e[:, ds],
                                in_=e[:, ds],
                                compare_op=ALU.is_ge,
                                fill=0.0,
                                base=0,
                                pattern=[[1, 128]],
                                channel_multiplier=-1,
                            )
                        for m in range(i_d, 4):
                            nden += 1
                            nc.tensor.matmul(
                                den_ps[:, m:m + 1],
                                e[:, m * 128:(m + 1) * 128],
                                ones_sb[:, 0:1],
                                start=(nden == 1),
                                stop=(nden == tden),
                                skip_group_check=(1 < nden < tden),
                            )
                        nc.tensor.matmul(
                            pctx[:, vs],
                            v_sb[:, kb, :],
                            e[:, vs],
                            start=(kb == 0),
                            stop=(kb == kmax - 1),
                        )
                    # reciprocal chain: cols -> row -> flatten
                    rcol = work.tile([128, 4], F32, tag="rcol")
                    nc.vector.reciprocal(rcol, den_ps)
                    rcolb = work.tile([128, 4], BF16, tag="rcolb")
                    nc.scalar.copy(rcolb, rcol)
                    prt = psL.tile([4, 128], BF16, tag="pl", name="rect")
                    nc.tensor.transpose(prt, rcolb, identb)
                    rrow4 = work.tile([4, 128], BF16, tag="rrow4")
                    nc.scalar.copy(rrow4, prt)
                    rflat = work.tile([1, 4, 128], BF16, tag="rflat")
                    nc.sync.dma_start(rflat, rrow4)
                    recs.append(rflat)
                    cu = cup.tile([128, CHUNK], BF16, tag="cu")
                    nc.vector.tensor_scalar_mul(cu, pctx, 1.0)
                    ctxs_u.append(cu)

                # normalize + quantize ctx -> fp8 hi/lo
                ctx_hi = cp.tile([128, G, CHUNK], FP8, tag="chi")
                ctx_lo = cp.tile([128, G, CHUNK], FP8, tag="clo")
                for g_ in range(G):
                    prb = psL.tile([128, CHUNK], F32, tag="pl", name="prb")
                    nc.tensor.matmul(prb, ones_sb[0:1, :], recs[g_][0:1],
                                     start=True, stop=True)
                    cbf = cp.tile([128, CHUNK], BF16, tag="cbf")
                    nc.vector.scalar_tensor_tensor(
                        out=cbf, in0=ctxs_u[g_], scalar=SCTX, in1=prb,
                        op0=ALU.mult, op1=ALU.mult,
                    )
                    nc.scalar.copy(ctx_hi[:, g_, :], cbf)
                    nc.gpsimd.tensor_sub(ctx_lo[:, g_, :], cbf,
                                         ctx_hi[:, g_, :])
                return ctx_hi, ctx_lo

            def emit_outproj(qb, ctx_hi, ctx_lo):
                for sm in range(4):
                    row = slice(qb * CHUNK + sm * 128, qb * CHUNK + (sm + 1) * 128)
                    ss = slice(sm * 128, (sm + 1) * 128)
                    for cb in range(4):
                        if qb == NCH - 1 and (sm * 4 + cb) % 2 == 1:
                            po = psA.tile([128, CHUNK], F32, tag="proj",
                                          name="po")
                        else:
                            po = psM.tile([128, 512], F32, tag="po", name="po")
                        n = 0  # 6 DR terms
                        for term in range(3):
                            ch, wo_ = ((ctx_hi, woh_sb), (ctx_lo, woh_sb),
                                       (ctx_hi, wol_sb))[term]
                            for gp in range(2):
                                n += 1
                                nc.tensor.matmul(
                                    po,
                                    ch[:, 2 * gp:2 * gp + 2, ss],
                                    wo_[:, gp, :, cb, :],
                                    start=(n == 1),
                                    stop=(n == 6),
                                    perf_mode=DR,
                                    skip_group_check=(1 < n < 6),
                                )  # gp-major
                        stage = stg.tile([128, 512], BF16, tag="stage")
                        nc.vector.tensor_scalar_mul(stage, po, INV_OUT)
                        nc.sync.dma_start(
                            outp[row, cb * 512:(cb + 1) * 512], stage
                        )

            # software-pipelined chunk loop: attention(qb) emits before
            # proj(qb+1), and outproj(qb) after it, so next-chunk projection
            # matmuls fill attention's exp-latency gaps and outproj fills
            # the following chunk's.
            qts = emit_proj(0)
            for qb in range(NCH):
                ctx_hi, ctx_lo = emit_attention(qb, qts)
                if qb + 1 < NCH:
                    qts = emit_proj(qb + 1)
                emit_outproj(qb, ctx_hi, ctx_lo)
    nc.compile()
    return nc


_PROGRAMS: dict[bool, bass.Bass] = {}


def get_program(has_bias: bool) -> bass.Bass:
    if has_bias not in _PROGRAMS:
        _PROGRAMS[has_bias] = build_program(has_bias)
    return _PROGRAMS[has_bias]


def _split_fp8(t, scale):
    """Return (hi, lo) fp8e4m3 arrays: t*scale = hi + lo (unscaled residual)."""
    ts = np.asarray(t, np.float32) * scale
    assert np.abs(ts).max() < 239.0, f"fp8 overflow {np.abs(ts).max()}"
    hi = ts.astype(NPF8)
    lo = (ts - hi.astype(np.float32)).astype(NPF8)
    return hi, lo


def _pack_pairs(w):
    """[H, M] -> [128, NJP, 2, M]: row (2j+i)*128+p -> [p, j, i, :]."""
    Hh, M = w.shape
    return np.ascontiguousarray(
        w.reshape(NJP, 2, 128, M).transpose(2, 0, 1, 3)
    )


def make_in_maps(x, ln_gamma, ln_beta, Wq, Wk, Wv, Wo):
    x = np.asarray(x, np.float64)
    g = np.asarray(ln_gamma, np.float64)
    be = np.asarray(ln_beta, np.float64)
    Wq = np.asarray(Wq, np.float64)
    Wk = np.asarray(Wk, np.float64)
    Wv = np.asarray(Wv, np.float64)
    Wo = np.asarray(Wo, np.float64)

    Wqg = Wq * g[:, None]
    Wkg = Wk * g[:, None]
    Wvg = Wv * g[:, None]
    bq_full = be @ Wq
    bk_full = be @ Wk
    bv_full = be @ Wv
    has_bias = bool(np.any(be != 0.0))

    # host LN stats
    mu = x.mean(-1, keepdims=True)  # [B, S, 1]
    var = ((x - mu) ** 2).mean(-1, keepdims=True)
    rstd = 1.0 / np.sqrt(var + EPS)
    xs = x * rstd  # x' = rstd * x
    brow = (-mu[..., 0] * rstd[..., 0]) * SX  # [B, S]

    # rope tables (halves duplicated)
    half = D // 2
    ts = MIN_WIN * (MAX_WIN / MIN_WIN) ** (
        2.0 * np.arange(half, dtype=np.float64) / D
    )
    ang = np.arange(S, dtype=np.float64)[None, :] / ts[:, None]
    cos_t = np.concatenate([np.cos(ang), np.cos(ang)], axis=0)
    sin_t = np.concatenate([np.sin(ang), np.sin(ang)], axis=0)

    prot = np.zeros((128, 128), np.float32)
    for m in range(half):
        prot[m + half, m] = -1.0
        prot[m, m + half] = 1.0

    # per-batch x' streams: [128, NJP, 2, S]
    xh_b, xl_b = [], []
    for b in range(B):
        xT = np.ascontiguousarray(xs[b].T)  # [H, S]
        hi, lo = _split_fp8(xT, SX)
        xh_b.append(_pack_pairs(hi.astype(np.float32)).astype(NPF8))
        xl_b.append(_pack_pairs(lo.astype(np.float32)).astype(NPF8))

    def wpack(Wg):
        hi, lo = _split_fp8(Wg, SW)
        return (_pack_pairs(hi.astype(np.float32)).astype(NPF8),
                _pack_pairs(lo.astype(np.float32)).astype(NPF8))

    in_maps = []
    for c in range(8):
        b, h = divmod(c, NKV)
        qs = slice(h * G * D, (h + 1) * G * D)
        ks = slice(h * D, (h + 1) * D)
        wqh, wql = wpack(Wqg[:, qs])
        wkh, wkl = wpack(Wkg[:, ks])
        wvh, wvl = wpack(Wvg[:, ks])
        # Wo rows for this core's 4 heads: [G*D, H]; head-pair packed:
        # [128(d), gp, i, cb, 512], row (2gp+i)*128+d of the slice
        Wo_c = Wo[qs, :] * SWO
        woh_f, wol_f = _split_fp8(Wo_c, 1.0)
        def wopack(wo8):
            w = wo8.astype(np.float32).reshape(2, 2, 128, 4, 512)
            return np.ascontiguousarray(
                w.transpose(2, 0, 1, 3, 4)
            ).astype(NPF8)
        in_maps.append({
            "xh": xh_b[b],
            "xl": xl_b[b],
            "wqh": wqh, "wql": wql,
            "wkh": wkh, "wkl": wkl,
            "wvh": wvh, "wvl": wvl,
            "woh": wopack(woh_f), "wol": wopack(wol_f),
            "wsq": (Wqg[:, qs].sum(0) * SW).astype(NPBF)[None, :],
            "wsk": (Wkg[:, ks].sum(0) * SW).astype(NPBF)[None, :],
            "wsv": (Wvg[:, ks].sum(0) * SW).astype(NPBF)[None, :],
            "brow": brow[b].astype(NPBF)[None, :],
            "bqr": (bq_full[qs] * SX * SW).astype(NPBF)[None, :],
            "bkr": (bk_full[ks] * SX * SW).astype(NPBF)[None, :],
            "bvr": (bv_full[ks] * SX * SW).astype(NPBF)[None, :],
            "ones_row": np.ones((1, S), np.float32).astype(NPBF),
            "cos_t": cos_t.astype(NPBF),
            "sin_t": sin_t.astype(NPBF),
            "prot": prot.astype(NPBF),
            "onesc": np.ones((128, 128), np.float32).astype(NPBF),
        })
    return in_maps, has_bias


def kernel(x, ln_gamma, ln_beta, Wq, Wk, Wv, Wo):
    from concourse.bass_utils import run_bass_kernel_spmd

    in_maps, has_bias = make_in_maps(x, ln_gamma, ln_beta, Wq, Wk, Wv, Wo)
    nc = get_program(has_bias)
    res = run_bass_kernel_spmd(nc, in_maps, core_ids=list(range(8)))
    outs = [np.asarray(m["outp"], np.float32) for m in res.results]
    out = np.empty((B, S, H), np.float32)
    for b in range(B):
        out[b] = (outs[NKV * b] + outs[NKV * b + 1]) + (
            outs[NKV * b + 2] + outs[NKV * b + 3]
        )
    return out
